# revision 12
# baseline (speedup 1.0000x reference)
"""Causal multi-head self-attention on 8 Trainium2 NeuronCores (Bass/Tile).

Problem (hardcoded shapes): x [2, 2048, 768] f32, 12 heads of dim 64.
    qkv = x @ Wqkv + bqkv ; per-head causal softmax(q k^T / 8) @ v ; out @ Wproj + bproj

Sharding: 8 cores = 2 batches x 4 head-groups (3 heads each). Each core computes
its heads' QKV, attention, and a partial output projection (its rows of Wproj).
Host sums the 4 partial projections per batch and adds bproj.

Per-core device dataflow (all layouts chosen so no on-device transposes of big
activations are ever needed except x -> xT, done as 96 128x128 PE matmuls
against the identity):
  xT[768,2048]   = transpose(x_b)                        (PE + DVE copies)
  qkT[384,2048]  = (Wqk^T x^T), heads on partitions      (PE, K=768 accum)
  v[2048,3*64+1] = x @ Wv, natural layout + ones column  (PE)
  per head, per key-tile kk (S^T layout: keys on partitions, queries free):
    S^T = k_h^T q_h / 8 (scale folded into Wq on host)   (PE, K=64)
    P^T = exp(S^T + causal_tri + key_mask_bias)          (ACT; no max-sub needed,
                                                          |scores| <= O(5) here)
    out_aug^T[65,2048] += v_aug^T P^T                    (PE, K=128; row 64 = colsum s)
  A^T = out^T[0:64] * (1/s broadcast via rank-1 matmul)  (DVE mult)
  y_partial[2048,768] = A @ Wproj_rows                   (PE, K=192)
"""
import os
import numpy as np

import concourse.bass as bass
import concourse.mybir as mybir
import concourse.tile as tile
from concourse import bacc
from concourse.bass_utils import run_bass_kernel_spmd
from concourse.masks import make_identity, make_lower_triangular

f32 = mybir.dt.float32
f32r = mybir.dt.float32r


def R(ap):
    """Bitcast an fp32 AP to float32r for full-rate PE matmuls."""
    return ap.bitcast(f32r)

T = 2048          # sequence length
H = 768           # model dim
NH_CORE = 3       # heads per core
HD = 64           # head dim
NT = T // 128     # 16 seq tiles
KH = H // 128     # 6 contraction chunks for H

_cache = {}
last_results = None


def _build():
    """Build the single-core Tile program (same program on all 8 cores)."""
    nc = bacc.Bacc("TRN2", target_bir_lowering=False, debug=False)

    x_d = nc.dram_tensor("xb", [T, H], f32, kind="ExternalInput")
    wqk_d = nc.dram_tensor("wqk", [H, 384], f32, kind="ExternalInput")
    wv_d = nc.dram_tensor("wv", [H, 256], f32, kind="ExternalInput")
    wp_d = nc.dram_tensor("wp", [192, H], f32, kind="ExternalInput")
    bqk_d = nc.dram_tensor("bqk", [128, 4], f32, kind="ExternalInput")
    bv_d = nc.dram_tensor("bv", [1, 256], f32, kind="ExternalInput")
    mask_d = nc.dram_tensor("maskcol", [128, NT], f32, kind="ExternalInput")
    y_d = nc.dram_tensor("y", [T, H], f32, kind="ExternalOutput")

    Exp = mybir.ActivationFunctionType.Exp
    Ident = mybir.ActivationFunctionType.Identity

    with tile.TileContext(nc) as tc:
        with (
            tc.tile_pool(name="singles", bufs=1) as singles,
            tc.tile_pool(name="big", bufs=1) as big,
            tc.tile_pool(name="xin", bufs=2) as xin,
            tc.tile_pool(name="pt", bufs=2) as ptp,
            tc.tile_pool(name="yout", bufs=2) as yout,
            tc.tile_pool(name="small", bufs=1) as small,
            tc.tile_pool(name="wstage", bufs=1) as wstage,
            tc.tile_pool(name="mm", bufs=4, space="PSUM") as mmp,
            tc.tile_pool(name="acc", bufs=1, space="PSUM") as accp,
        ):
            # ---- constants / weights ----
            ident = singles.tile([128, 128], f32)
            make_identity(nc, ident[:])
            tri = singles.tile([128, 128], f32)
            make_lower_triangular(nc, tri[:], val=-1e9, diag=False)
            ones = singles.tile([1, 128], f32)
            nc.vector.memset(ones[:], 1.0)
            onesr = singles.tile([1, 128], f32)
            nc.vector.tensor_copy(R(onesr[:]), ones[:])
            ones128 = singles.tile([128, 1], f32)
            nc.vector.memset(ones128[:], 1.0)
            zeros128 = singles.tile([128, 1], f32)
            nc.vector.memset(zeros128[:], 0.0)

            # DMA-loaded matmul operands must be rounded to fp32r by a
            # compute engine before an fp32r matmul may consume them: stage
            # each load through a scratch tile, round on DVE into the real one.
            def load_rounded(dst, src_ap):
                p = dst.shape[0]
                free = 1
                for d in dst.shape[1:]:
                    free *= d
                st = wstage.tile([128, KH * 384], f32, tag="wstage")
                sv = st[0:p, 0:free]
                if len(dst.shape) == 3:
                    sv = sv.rearrange("p (a m) -> p a m", a=dst.shape[1])
                nc.sync.dma_start(out=sv, in_=src_ap)
                nc.vector.tensor_copy(R(dst[:]), sv)

            wqk = singles.tile([128, KH, 384], f32)
            load_rounded(wqk, wqk_d.ap().rearrange("(a p) m -> p a m", p=128))
            wv = singles.tile([128, KH, 256], f32)
            load_rounded(wv, wv_d.ap().rearrange("(a p) m -> p a m", p=128))
            wp0 = singles.tile([128, H], f32)
            load_rounded(wp0, wp_d.ap()[0:128, :])
            wp1 = singles.tile([64, H], f32)
            load_rounded(wp1, wp_d.ap()[128:192, :])
            bv = singles.tile([1, 256], f32)
            load_rounded(bv, bv_d.ap())
            bqk = singles.tile([128, 4], f32)
            nc.sync.dma_start(out=bqk[:], in_=bqk_d.ap())
            maskcol = singles.tile([128, NT], f32)
            nc.sync.dma_start(out=maskcol[:], in_=mask_d.ap())

            # ---- phase 1: load x and transpose into xT [768, 2048] ----
            xT = big.tile([128, KH, T], f32)
            for t in range(NT):
                xt = xin.tile([128, H], f32)
                nc.sync.dma_start(out=xt[:], in_=x_d.ap()[t * 128:(t + 1) * 128, :])
                # 6 transposes per x tile; batch 3 per PSUM bank -> 2 DVE copies
                for g in range(2):
                    ps = mmp.tile([128, 512], f32, tag="mm")
                    for j in range(3):
                        hh = g * 3 + j
                        nc.tensor.matmul(ps[:, j * 128:(j + 1) * 128],
                                         lhsT=xt[:, hh * 128:(hh + 1) * 128],
                                         rhs=ident[:], start=True, stop=True,
                                         is_transpose=True)
                    nc.vector.tensor_copy(
                        R(xT[:, g * 3:(g + 1) * 3, t * 128:(t + 1) * 128]),
                        ps[:, 0:384].rearrange("p (j c) -> p j c", j=3))

            # ---- phase 2: qkT [384, 2048] (chunks {q0q1, k0k1, q2, k2}) ----
            qkA = big.tile([128, T], f32)   # q0 (0:64), q1 (64:128)
            qkB = big.tile([128, T], f32)   # k0, k1
            qkC = big.tile([64, T], f32)    # q2
            qkD = big.tile([64, T], f32)    # k2
            chunks = [(qkA, 0, 128), (qkB, 128, 128), (qkC, 256, 64), (qkD, 320, 64)]
            for mi, (dst, moff, mp) in enumerate(chunks):
                for n in range(4):
                    ns = slice(n * 512, (n + 1) * 512)
                    ps = mmp.tile([128, 512], f32, tag="mm")
                    for k in range(KH):
                        nc.tensor.matmul(ps[0:mp, :], lhsT=R(wqk[:, k, moff:moff + mp]),
                                         rhs=R(xT[:, k, ns]), start=(k == 0), stop=(k == KH - 1))
                    nc.scalar.activation(out=R(dst[:, ns]), in_=ps[0:mp, :], func=Ident,
                                         bias=bqk[0:mp, mi:mi + 1], scale=1.0)

            # ---- phase 2b: v natural [2048, 3*65] with ones column ----
            vsb = big.tile([128, NT, NH_CORE, HD + 1], f32)
            nc.vector.tensor_copy(R(vsb[:, :, :, HD:HD + 1]),
                                  ones128[:, 0:1].to_broadcast((128, NT, NH_CORE, 1)))
            for t in range(NT):
                ps = mmp.tile([128, 256], f32, tag="mm")
                for k in range(KH):
                    nc.tensor.matmul(ps[:], lhsT=R(xT[:, k, t * 128:(t + 1) * 128]),
                                     rhs=R(wv[:, k, :]), start=(k == 0), stop=False)
                nc.tensor.matmul(ps[:], lhsT=R(onesr[:]), rhs=R(bv[:]), start=False, stop=True)
                nc.vector.tensor_copy(R(vsb[:, t, :, 0:HD]),
                                      ps[:, 0:192].rearrange("p (h d) -> p h d", h=NH_CORE))

            # ---- phase 3: attention per head ----
            qk_of = [(qkA, 0), (qkA, 64), (qkC, 0)]
            kk_of = [(qkB, 0), (qkB, 64), (qkD, 0)]
            atA = big.tile([128, T], f32)   # A^T for h0 (0:64), h1 (64:128)
            atB = big.tile([64, T], f32)    # h2
            at_of = [(atA, 0), (atA, 64), (atB, 0)]

            for h in range(NH_CORE):
                qt_t, qt_o = qk_of[h]
                kt_t, kt_o = kk_of[h]
                qT = qt_t[qt_o:qt_o + HD, :]
                kT = kt_t[kt_o:kt_o + HD, :]
                oacc = accp.tile([HD + 1, T], f32)
                for kk in range(NT):
                    qc0 = kk // 4
                    qlo = kk * 128
                    pt = ptp.tile([128, T], f32)
                    for qc in range(qc0, 4):
                        qs = slice(qc * 512, (qc + 1) * 512)
                        sps = mmp.tile([128, 512], f32, tag="mm")
                        nc.tensor.matmul(sps[:], lhsT=R(kT[:, qlo:qlo + 128]),
                                         rhs=R(qT[:, qs]), start=True, stop=True)
                        if qc == qc0:
                            off = qlo - qc0 * 512
                            nc.vector.tensor_tensor(sps[:, off:off + 128],
                                                    sps[:, off:off + 128], tri[:],
                                                    mybir.AluOpType.add)
                            if off > 0:
                                nc.vector.tensor_copy(R(pt[:, qc * 512:qc * 512 + off]),
                                                      zeros128[:, 0:1].to_broadcast((128, off)))
                            nc.scalar.activation(out=R(pt[:, qc * 512 + off:(qc + 1) * 512]),
                                                 in_=sps[:, off:512], func=Exp,
                                                 bias=maskcol[:, kk:kk + 1], scale=1.0)
                        else:
                            nc.scalar.activation(out=R(pt[:, qs]), in_=sps[:], func=Exp,
                                                 bias=maskcol[:, kk:kk + 1], scale=1.0)
                    for qc in range(qc0, 4):
                        qs = slice(qc * 512, (qc + 1) * 512)
                        nc.tensor.matmul(oacc[:, qs], lhsT=R(vsb[:, kk, h, :]),
                                         rhs=R(pt[:, qs]), start=(kk == 0),
                                         stop=(kk == 4 * qc + 3))
                # normalize: A^T = out^T[0:64] * (1/s), s = row 64
                srow = small.tile([1, T], f32)
                nc.vector.tensor_copy(srow[:], oacc[HD:HD + 1, :])
                at_t, at_o = at_of[h]
                rbs = small.tile([64, T], f32, tag="rbs")
                nc.gpsimd.partition_broadcast(rbs[:], srow[:])
                nc.vector.reciprocal(rbs[:], rbs[:])
                nc.vector.tensor_tensor(R(at_t[at_o:at_o + HD, :]),
                                        oacc[0:HD, :], rbs[:],
                                        mybir.AluOpType.mult)

            # ---- phase 4: projection y = A @ Wp ----
            for t in range(NT):
                ts = slice(t * 128, (t + 1) * 128)
                yt = yout.tile([128, H], f32)
                for n in range(2):
                    ns = slice(n * 384, (n + 1) * 384)
                    yps = mmp.tile([128, 384], f32, tag="mm")
                    nc.tensor.matmul(yps[:], lhsT=R(atA[:, ts]), rhs=R(wp0[:, ns]),
                                     start=True, stop=False)
                    nc.tensor.matmul(yps[:], lhsT=R(atB[:, ts]), rhs=R(wp1[:, ns]),
                                     start=False, stop=True)
                    nc.vector.tensor_copy(yt[:, ns], yps[:])
                nc.sync.dma_start(out=y_d.ap()[ts, :], in_=yt[:])

    nc.compile()
    return nc


def kernel(x, attn_mask, Wqkv, bqkv, Wproj, bproj):
    global last_results
    x = np.asarray(x, dtype=np.float32)
    attn_mask = np.asarray(attn_mask)
    Wqkv = np.asarray(Wqkv, dtype=np.float32)
    bqkv = np.asarray(bqkv, dtype=np.float32)
    Wproj = np.asarray(Wproj, dtype=np.float32)
    bproj = np.asarray(bproj, dtype=np.float32)

    if "nc" not in _cache:
        _cache["nc"] = _build()
    nc = _cache["nc"]

    in_maps = []
    for c in range(8):
        b, g = c // 4, c % 4
        cs = slice(64 * 3 * g, 64 * 3 * g + 192)
        wq = Wqkv[:, 0:768][:, cs] * 0.125
        bq = bqkv[0:768][cs] * 0.125
        wk = Wqkv[:, 768:1536][:, cs]
        bk = bqkv[768:1536][cs]
        wv = Wqkv[:, 1536:2304][:, cs]
        bv = bqkv[1536:2304][cs]
        wqk = np.concatenate([wq[:, 0:128], wk[:, 0:128], wq[:, 128:192], wk[:, 128:192]], axis=1)
        bqk = np.zeros((128, 4), np.float32)
        bqk[:, 0] = bq[0:128]
        bqk[:, 1] = bk[0:128]
        bqk[0:64, 2] = bq[128:192]
        bqk[0:64, 3] = bk[128:192]
        maskcol = np.ascontiguousarray(
            (attn_mask[b].astype(np.float32).reshape(NT, 128).T - 1.0) * 1e9)
        wv_pad = np.zeros((768, 256), np.float32)
        wv_pad[:, 0:192] = wv
        bv_pad = np.zeros((1, 256), np.float32)
        bv_pad[0, 0:192] = bv
        in_maps.append({
            "xb": np.ascontiguousarray(x[b]),
            "wqk": np.ascontiguousarray(wqk),
            "wv": wv_pad,
            "wp": np.ascontiguousarray(Wproj[cs, :]),
            "bqk": bqk,
            "bv": bv_pad,
            "maskcol": maskcol,
        })

    trace = bool(int(os.environ.get("KERNEL_TRACE", "0")))
    res = run_bass_kernel_spmd(nc, in_maps, core_ids=list(range(8)), trace=trace)
    last_results = res

    parts = [res.results[c]["y"] for c in range(8)]
    out = np.stack([
        parts[0] + parts[1] + parts[2] + parts[3],
        parts[4] + parts[5] + parts[6] + parts[7],
    ]).astype(np.float32) + bproj.astype(np.float32)
    return out.astype(np.float32)


# revision 19
# speedup vs baseline: 1.0875x; 1.0875x over previous
"""Causal multi-head self-attention on 8 Trainium2 NeuronCores (Bass/Tile).

Problem (hardcoded shapes): x [2, 2048, 768] f32, 12 heads of dim 64.
    qkv = x @ Wqkv + bqkv ; per-head causal softmax(q k^T / 8) @ v ; out @ Wproj + bproj

Sharding: 8 cores = 2 batches x 4 head-groups (3 heads each). Each core computes
its heads' QKV, attention, and a partial output projection (its rows of Wproj).
Host sums the 4 partial projections per batch and adds bproj.

Per-core device dataflow (all layouts chosen so no on-device transposes of big
activations are ever needed except x -> xT, done as 96 128x128 PE matmuls
against the identity):
  xT[768,2048]   = transpose(x_b)                        (PE + DVE copies)
  qkT[384,2048]  = (Wqk^T x^T), heads on partitions      (PE, K=768 accum)
  v[2048,3*64+1] = x @ Wv, natural layout + ones column  (PE)
  per head, per key-tile kk (S^T layout: keys on partitions, queries free):
    S^T = k_h^T q_h / 8 (scale folded into Wq on host)   (PE, K=64)
    P^T = exp(S^T + causal_tri + key_mask_bias)          (ACT; no max-sub needed,
                                                          |scores| <= O(5) here)
    out_aug^T[65,2048] += v_aug^T P^T                    (PE, K=128; row 64 = colsum s)
  A^T = out^T[0:64] * (1/s broadcast via rank-1 matmul)  (DVE mult)
  y_partial[2048,768] = A @ Wproj_rows                   (PE, K=192)
"""
import os
import numpy as np

import concourse.bass as bass
import concourse.mybir as mybir
import concourse.tile as tile
from concourse import bacc
from concourse.bass_utils import run_bass_kernel_spmd
from concourse.masks import make_identity, make_upper_triangular

f32 = mybir.dt.float32
f32r = mybir.dt.float32r


def R(ap):
    """Bitcast an fp32 AP to float32r for full-rate PE matmuls."""
    return ap.bitcast(f32r)

T = 2048          # sequence length
H = 768           # model dim
NH_CORE = 3       # heads per core
HD = 64           # head dim
NT = T // 128     # 16 seq tiles
KH = H // 128     # 6 contraction chunks for H

_cache = {}
last_results = None


def _build():
    """Build the single-core Tile program (same program on all 8 cores)."""
    nc = bacc.Bacc("TRN2", target_bir_lowering=False, debug=False)

    x_d = nc.dram_tensor("xb", [T, H], f32, kind="ExternalInput")
    wqk_d = nc.dram_tensor("wqk", [H, 384], f32, kind="ExternalInput")
    wv_d = nc.dram_tensor("wv", [H, 256], f32, kind="ExternalInput")
    wp_d = nc.dram_tensor("wp", [192, H], f32, kind="ExternalInput")
    bqk_d = nc.dram_tensor("bqk", [128, 4], f32, kind="ExternalInput")
    bv_d = nc.dram_tensor("bv", [1, 256], f32, kind="ExternalInput")
    mask_d = nc.dram_tensor("maskcol", [128, NT], f32, kind="ExternalInput")
    y_d = nc.dram_tensor("y", [T, H], f32, kind="ExternalOutput")

    Exp = mybir.ActivationFunctionType.Exp
    Ident = mybir.ActivationFunctionType.Identity

    with tile.TileContext(nc) as tc:
        with (
            tc.tile_pool(name="singles", bufs=1) as singles,
            tc.tile_pool(name="big", bufs=1) as big,
            tc.tile_pool(name="xin", bufs=2) as xin,
            tc.tile_pool(name="pt", bufs=4) as ptp,
            tc.tile_pool(name="yout", bufs=2) as yout,
            tc.tile_pool(name="small", bufs=2) as small,
            tc.tile_pool(name="wstage", bufs=1) as wstage,
            tc.tile_pool(name="mm", bufs=2, space="PSUM") as mmp,
            tc.tile_pool(name="acc", bufs=3, space="PSUM") as accp,
        ):
            # ---- constants / weights ----
            ident = singles.tile([128, 128], f32)
            make_identity(nc, ident[:])
            tri01 = singles.tile([128, 128], f32)
            make_upper_triangular(nc, tri01[:], val=1.0, diag=True)
            ones = singles.tile([1, 128], f32)
            nc.vector.memset(ones[:], 1.0)
            onesr = singles.tile([1, 128], f32)
            nc.vector.tensor_copy(R(onesr[:]), ones[:])
            ones128 = singles.tile([128, 1], f32)
            nc.vector.memset(ones128[:], 1.0)
            zeros128 = singles.tile([128, 1], f32)
            nc.vector.memset(zeros128[:], 0.0)
            ones_bc = singles.tile([64, 512], f32)
            nc.vector.memset(ones_bc[:], 1.0)

            # DMA-loaded matmul operands must be rounded to fp32r by a
            # compute engine before an fp32r matmul may consume them: stage
            # each load through a scratch tile, round on DVE into the real one.
            def load_rounded(dst, src_ap):
                p = dst.shape[0]
                free = 1
                for d in dst.shape[1:]:
                    free *= d
                st = wstage.tile([128, KH * 384], f32, tag="wstage")
                sv = st[0:p, 0:free]
                if len(dst.shape) == 3:
                    sv = sv.rearrange("p (a m) -> p a m", a=dst.shape[1])
                nc.sync.dma_start(out=sv, in_=src_ap)
                nc.vector.tensor_copy(R(dst[:]), sv)

            wqk = singles.tile([128, KH, 384], f32)
            load_rounded(wqk, wqk_d.ap().rearrange("(a p) m -> p a m", p=128))
            wv = singles.tile([128, KH, 256], f32)
            load_rounded(wv, wv_d.ap().rearrange("(a p) m -> p a m", p=128))
            wp0 = singles.tile([128, H], f32)
            load_rounded(wp0, wp_d.ap()[0:128, :])
            wp1 = singles.tile([64, H], f32)
            load_rounded(wp1, wp_d.ap()[128:192, :])
            bv = singles.tile([1, 256], f32)
            load_rounded(bv, bv_d.ap())
            bqk = singles.tile([128, 4], f32)
            nc.sync.dma_start(out=bqk[:], in_=bqk_d.ap())
            maskcol = singles.tile([128, NT], f32)
            nc.sync.dma_start(out=maskcol[:], in_=mask_d.ap())

            # ---- phase 1: load x and transpose into xT [768, 2048] ----
            xT = big.tile([128, KH, T], f32)
            for t in range(NT):
                xt = xin.tile([128, H], f32)
                nc.sync.dma_start(out=xt[:], in_=x_d.ap()[t * 128:(t + 1) * 128, :])
                # 6 transposes per x tile; batch 3 per PSUM bank -> 2 DVE copies
                for g in range(2):
                    ps = mmp.tile([128, 512], f32, tag="mm")
                    for j in range(3):
                        hh = g * 3 + j
                        nc.tensor.matmul(ps[:, j * 128:(j + 1) * 128],
                                         lhsT=xt[:, hh * 128:(hh + 1) * 128],
                                         rhs=ident[:], start=True, stop=True,
                                         is_transpose=True)
                    nc.vector.tensor_copy(
                        R(xT[:, g * 3:(g + 1) * 3, t * 128:(t + 1) * 128]),
                        ps[:, 0:384].rearrange("p (j c) -> p j c", j=3))

            # ---- phase 2: qkT [384, 2048] (chunks {q0q1, k0k1, q2, k2}) ----
            qkA = big.tile([128, T], f32)   # q0 (0:64), q1 (64:128)
            qkB = big.tile([128, T], f32)   # k0, k1
            qkC = big.tile([64, T], f32)    # q2
            qkD = big.tile([64, T], f32)    # k2
            chunks = [(qkA, 0, 128), (qkB, 128, 128), (qkC, 256, 64), (qkD, 320, 64)]
            for mi, (dst, moff, mp) in enumerate(chunks):
                for n in range(4):
                    ns = slice(n * 512, (n + 1) * 512)
                    ps = mmp.tile([128, 512], f32, tag="mm")
                    for k in range(KH):
                        nc.tensor.matmul(ps[0:mp, :], lhsT=R(wqk[:, k, moff:moff + mp]),
                                         rhs=R(xT[:, k, ns]), start=(k == 0), stop=(k == KH - 1))
                    nc.scalar.activation(out=R(dst[:, ns]), in_=ps[0:mp, :], func=Ident,
                                         bias=bqk[0:mp, mi:mi + 1], scale=1.0)

            # ---- phase 2b: v natural [2048, 3*65] with ones column ----
            vsb = big.tile([128, NT, NH_CORE, HD + 1], f32)
            nc.vector.tensor_copy(R(vsb[:, :, :, HD:HD + 1]),
                                  ones128[:, 0:1].to_broadcast((128, NT, NH_CORE, 1)))
            for t in range(NT):
                ps = mmp.tile([128, 256], f32, tag="mm")
                for k in range(KH):
                    nc.tensor.matmul(ps[:], lhsT=R(xT[:, k, t * 128:(t + 1) * 128]),
                                     rhs=R(wv[:, k, :]), start=(k == 0), stop=False)
                nc.tensor.matmul(ps[:], lhsT=R(onesr[:]), rhs=R(bv[:]), start=False, stop=True)
                nc.vector.tensor_copy(R(vsb[:, t, :, 0:HD]),
                                      ps[:, 0:192].rearrange("p (h d) -> p h d", h=NH_CORE))

            # ---- phase 3: attention, query-chunk outer, heads interleaved ----
            # h0 scores use PE rows 0-63, h1 rows 64-127 (adjacent emission ->
            # the PE runs them concurrently); h2 follows on rows 0-63.
            qk_of = [(qkA, 0), (qkA, 64), (qkC, 0)]
            kk_of = [(qkB, 0), (qkB, 64), (qkD, 0)]
            atA = big.tile([128, T], f32)   # A^T for h0 (0:64), h1 (64:128)
            atB = big.tile([64, T], f32)    # h2
            at_of = [(atA, 0), (atA, 64), (atB, 0)]
            qTs = [t_[o:o + HD, :] for (t_, o) in qk_of]
            kTs = [t_[o:o + HD, :] for (t_, o) in kk_of]

            for qc in range(4):
                base = qc * 512
                oaccs = [accp.tile([HD + 1, 512], f32, tag="acc", name=f"oacc{_h}")
                          for _h in range(3)]
                for kk in range(4 * qc + 4):
                    qlo = kk * 128
                    off = max(0, qlo - base)
                    w = 512 - off
                    s2 = mmp.tile([128, 1024], f32, tag="mm")
                    nc.tensor.matmul(s2[:, off:512], lhsT=R(kTs[0][:, qlo:qlo + 128]),
                                     rhs=R(qTs[0][:, base + off:base + 512]),
                                     start=True, stop=True)
                    nc.tensor.matmul(s2[:, 512 + off:1024],
                                     lhsT=R(kTs[1][:, qlo:qlo + 128]),
                                     rhs=R(qTs[1][:, base + off:base + 512]),
                                     start=True, stop=True)
                    s1 = mmp.tile([128, 1024], f32, tag="mm")
                    nc.tensor.matmul(s1[:, off:512], lhsT=R(kTs[2][:, qlo:qlo + 128]),
                                     rhs=R(qTs[2][:, base + off:base + 512]),
                                     start=True, stop=True)
                    pt2 = ptp.tile([128, 1024], f32, tag="pt")
                    pt1 = ptp.tile([128, 1024], f32, tag="pt")
                    if off == 0:
                        nc.scalar.activation(out=R(pt2[:]), in_=s2[:], func=Exp,
                                             bias=maskcol[:, kk:kk + 1], scale=1.0)
                    else:
                        nc.scalar.activation(out=R(pt2[:, off:512]), in_=s2[:, off:512],
                                             func=Exp, bias=maskcol[:, kk:kk + 1],
                                             scale=1.0)
                        nc.scalar.activation(out=R(pt2[:, 512 + off:1024]),
                                             in_=s2[:, 512 + off:1024], func=Exp,
                                             bias=maskcol[:, kk:kk + 1], scale=1.0)
                    nc.scalar.activation(out=R(pt1[:, off:512]), in_=s1[:, off:512],
                                         func=Exp, bias=maskcol[:, kk:kk + 1],
                                         scale=1.0)
                    if qlo >= base:
                        # diagonal block: zero keys > query
                        d = off
                        for pt, o2 in ((pt2, 0), (pt2, 512), (pt1, 0)):
                            nc.vector.tensor_tensor(R(pt[:, o2 + d:o2 + d + 128]),
                                                    pt[:, o2 + d:o2 + d + 128],
                                                    tri01[:], mybir.AluOpType.mult)
                    for h, (pt, o2) in enumerate(((pt2, 0), (pt2, 512), (pt1, 0))):
                        nc.tensor.matmul(oaccs[h][0:HD + 1, off:512],
                                         lhsT=R(vsb[:, kk, h, :]),
                                         rhs=R(pt[:, o2 + off:o2 + 512]),
                                         start=(kk == 0), stop=(kk == 4 * qc + 3),
                                         skip_group_check=True)
                # normalize: A^T[:, base:base+512] = out^T[0:64] / s (s = row 64)
                # gpsimd does broadcast + reciprocal-via-divide (SBUF only);
                # DVE does the final fp32r multiply from PSUM.
                for h in range(3):
                    srow = small.tile([1, 512], f32, tag="srow")
                    nc.vector.tensor_copy(srow[:], oaccs[h][HD:HD + 1, :])
                    rrow = small.tile([1, 512], f32, tag="rrow")
                    nc.vector.reciprocal_approx_fast(rrow[:], srow[:])
                    rbs = small.tile([64, 512], f32, tag="rbs")
                    nc.gpsimd.partition_broadcast(rbs[:], rrow[:])
                    at_t, at_o = at_of[h]
                    nc.vector.tensor_tensor(R(at_t[at_o:at_o + HD, base:base + 512]),
                                            oaccs[h][0:HD, :], rbs[:],
                                            mybir.AluOpType.mult)

            # ---- phase 4: projection y = A @ Wp ----
            for t in range(NT):
                ts = slice(t * 128, (t + 1) * 128)
                yt = yout.tile([128, H], f32)
                for n in range(2):
                    ns = slice(n * 384, (n + 1) * 384)
                    yps = mmp.tile([128, 384], f32, tag="mm")
                    nc.tensor.matmul(yps[:], lhsT=R(atA[:, ts]), rhs=R(wp0[:, ns]),
                                     start=True, stop=False)
                    nc.tensor.matmul(yps[:], lhsT=R(atB[:, ts]), rhs=R(wp1[:, ns]),
                                     start=False, stop=True)
                    nc.vector.tensor_copy(yt[:, ns], yps[:])
                nc.sync.dma_start(out=y_d.ap()[ts, :], in_=yt[:])

    nc.compile()
    return nc


def kernel(x, attn_mask, Wqkv, bqkv, Wproj, bproj):
    global last_results
    x = np.asarray(x, dtype=np.float32)
    attn_mask = np.asarray(attn_mask)
    Wqkv = np.asarray(Wqkv, dtype=np.float32)
    bqkv = np.asarray(bqkv, dtype=np.float32)
    Wproj = np.asarray(Wproj, dtype=np.float32)
    bproj = np.asarray(bproj, dtype=np.float32)

    if "nc" not in _cache:
        _cache["nc"] = _build()
    nc = _cache["nc"]

    in_maps = []
    for c in range(8):
        b, g = c // 4, c % 4
        cs = slice(64 * 3 * g, 64 * 3 * g + 192)
        wq = Wqkv[:, 0:768][:, cs] * 0.125
        bq = bqkv[0:768][cs] * 0.125
        wk = Wqkv[:, 768:1536][:, cs]
        bk = bqkv[768:1536][cs]
        wv = Wqkv[:, 1536:2304][:, cs]
        bv = bqkv[1536:2304][cs]
        wqk = np.concatenate([wq[:, 0:128], wk[:, 0:128], wq[:, 128:192], wk[:, 128:192]], axis=1)
        bqk = np.zeros((128, 4), np.float32)
        bqk[:, 0] = bq[0:128]
        bqk[:, 1] = bk[0:128]
        bqk[0:64, 2] = bq[128:192]
        bqk[0:64, 3] = bk[128:192]
        maskcol = np.ascontiguousarray(
            (attn_mask[b].astype(np.float32).reshape(NT, 128).T - 1.0) * 1e9)
        wv_pad = np.zeros((768, 256), np.float32)
        wv_pad[:, 0:192] = wv
        bv_pad = np.zeros((1, 256), np.float32)
        bv_pad[0, 0:192] = bv
        in_maps.append({
            "xb": np.ascontiguousarray(x[b]),
            "wqk": np.ascontiguousarray(wqk),
            "wv": wv_pad,
            "wp": np.ascontiguousarray(Wproj[cs, :]),
            "bqk": bqk,
            "bv": bv_pad,
            "maskcol": maskcol,
        })

    trace = bool(int(os.environ.get("KERNEL_TRACE", "0")))
    res = run_bass_kernel_spmd(nc, in_maps, core_ids=list(range(8)), trace=trace)
    last_results = res

    parts = [res.results[c]["y"] for c in range(8)]
    out = np.stack([
        parts[0] + parts[1] + parts[2] + parts[3],
        parts[4] + parts[5] + parts[6] + parts[7],
    ]).astype(np.float32) + bproj.astype(np.float32)
    return out.astype(np.float32)


# revision 21
# speedup vs baseline: 1.2297x; 1.1308x over previous
"""Causal multi-head self-attention on 8 Trainium2 NeuronCores (Bass/Tile).

Problem (hardcoded shapes): x [2, 2048, 768] f32, 12 heads of dim 64.
    qkv = x @ Wqkv + bqkv ; per-head causal softmax(q k^T / 8) @ v ; out @ Wproj + bproj

Sharding: 8 cores = 2 batches x 4 head-groups (3 heads each). Each core computes
its heads' QKV, attention, and a partial output projection (its rows of Wproj).
Host sums the 4 partial projections per batch and adds bproj.

Per-core device dataflow (all layouts chosen so no on-device transposes of big
activations are ever needed except x -> xT, done as 96 128x128 PE matmuls
against the identity):
  xT[768,2048]   = transpose(x_b)                        (PE + DVE copies)
  qkT[384,2048]  = (Wqk^T x^T), heads on partitions      (PE, K=768 accum)
  v[2048,3*64+1] = x @ Wv, natural layout + ones column  (PE)
  per head, per key-tile kk (S^T layout: keys on partitions, queries free):
    S^T = k_h^T q_h / 8 (scale folded into Wq on host)   (PE, K=64)
    P^T = exp(S^T + causal_tri + key_mask_bias)          (ACT; no max-sub needed,
                                                          |scores| <= O(5) here)
    out_aug^T[65,2048] += v_aug^T P^T                    (PE, K=128; row 64 = colsum s)
  A^T = out^T[0:64] * (1/s broadcast via rank-1 matmul)  (DVE mult)
  y_partial[2048,768] = A @ Wproj_rows                   (PE, K=192)
"""
import os
import numpy as np

import concourse.bass as bass
import concourse.mybir as mybir
import concourse.tile as tile
from concourse import bacc
from concourse.bass_utils import run_bass_kernel_spmd
from concourse.masks import make_identity, make_upper_triangular

f32 = mybir.dt.float32
f32r = mybir.dt.float32r


def R(ap):
    """Bitcast an fp32 AP to float32r for full-rate PE matmuls."""
    return ap.bitcast(f32r)

T = 2048          # sequence length
H = 768           # model dim
NH_CORE = 3       # heads per core
HD = 64           # head dim
NT = T // 128     # 16 seq tiles
KH = H // 128     # 6 contraction chunks for H

_cache = {}
last_results = None


def _build():
    """Build the single-core Tile program (same program on all 8 cores)."""
    nc = bacc.Bacc("TRN2", target_bir_lowering=False, debug=False)

    x_d = nc.dram_tensor("xb", [T, H], f32, kind="ExternalInput")
    wqk_d = nc.dram_tensor("wqk", [H, 384], f32, kind="ExternalInput")
    wv_d = nc.dram_tensor("wv", [H, 256], f32, kind="ExternalInput")
    wp_d = nc.dram_tensor("wp", [192, H], f32, kind="ExternalInput")
    bqk_d = nc.dram_tensor("bqk", [128, 4], f32, kind="ExternalInput")
    bv_d = nc.dram_tensor("bv", [1, 256], f32, kind="ExternalInput")
    mask_d = nc.dram_tensor("maskcol", [128, NT], f32, kind="ExternalInput")
    y_d = nc.dram_tensor("y", [T, H], f32, kind="ExternalOutput")

    Exp = mybir.ActivationFunctionType.Exp
    Ident = mybir.ActivationFunctionType.Identity

    with tile.TileContext(nc) as tc:
        with (
            tc.tile_pool(name="singles", bufs=1) as singles,
            tc.tile_pool(name="big", bufs=1) as big,
            tc.tile_pool(name="xin", bufs=2) as xin,
            tc.tile_pool(name="pt", bufs=4) as ptp,
            tc.tile_pool(name="yout", bufs=2) as yout,
            tc.tile_pool(name="small", bufs=2) as small,
            tc.tile_pool(name="wstage", bufs=1) as wstage,
            tc.tile_pool(name="mm", bufs=2, space="PSUM") as mmp,
            tc.tile_pool(name="acc", bufs=3, space="PSUM") as accp,
        ):
            # ---- constants / weights ----
            ident = singles.tile([128, 128], f32)
            make_identity(nc, ident[:])
            tri01 = singles.tile([128, 128], f32)
            make_upper_triangular(nc, tri01[:], val=1.0, diag=True)
            ones = singles.tile([1, 128], f32)
            nc.vector.memset(ones[:], 1.0)
            onesr = singles.tile([1, 128], f32)
            nc.vector.tensor_copy(R(onesr[:]), ones[:])
            ones128 = singles.tile([128, 1], f32)
            nc.vector.memset(ones128[:], 1.0)
            zeros128 = singles.tile([128, 1], f32)
            nc.vector.memset(zeros128[:], 0.0)
            ones_bc = singles.tile([64, 512], f32)
            nc.vector.memset(ones_bc[:], 1.0)

            # DMA-loaded matmul operands must be rounded to fp32r by a
            # compute engine before an fp32r matmul may consume them: stage
            # each load through a scratch tile, round on DVE into the real one.
            def load_rounded(dst, src_ap):
                p = dst.shape[0]
                free = 1
                for d in dst.shape[1:]:
                    free *= d
                st = wstage.tile([128, KH * 384], f32, tag="wstage")
                sv = st[0:p, 0:free]
                if len(dst.shape) == 3:
                    sv = sv.rearrange("p (a m) -> p a m", a=dst.shape[1])
                nc.sync.dma_start(out=sv, in_=src_ap)
                nc.vector.tensor_copy(R(dst[:]), sv)

            wqk = singles.tile([128, KH, 384], f32)
            load_rounded(wqk, wqk_d.ap().rearrange("(a p) m -> p a m", p=128))
            wv = singles.tile([128, KH, 256], f32)
            load_rounded(wv, wv_d.ap().rearrange("(a p) m -> p a m", p=128))
            wp0 = singles.tile([128, H], f32)
            load_rounded(wp0, wp_d.ap()[0:128, :])
            wp1 = singles.tile([64, H], f32)
            load_rounded(wp1, wp_d.ap()[128:192, :])
            bv = singles.tile([1, 256], f32)
            load_rounded(bv, bv_d.ap())
            bqk = singles.tile([128, 4], f32)
            nc.sync.dma_start(out=bqk[:], in_=bqk_d.ap())
            maskcol = singles.tile([128, NT], f32)
            nc.sync.dma_start(out=maskcol[:], in_=mask_d.ap())

            # ---- phase 1: load x and transpose into xT [768, 2048] ----
            xT = big.tile([128, KH, T], f32)
            for t in range(NT):
                xt = xin.tile([128, H], f32)
                nc.sync.dma_start(out=xt[:], in_=x_d.ap()[t * 128:(t + 1) * 128, :])
                # 6 transposes per x tile; batch 3 per PSUM bank -> 2 DVE copies
                for g in range(2):
                    ps = mmp.tile([128, 512], f32, tag="mm")
                    for j in range(3):
                        hh = g * 3 + j
                        nc.tensor.matmul(ps[:, j * 128:(j + 1) * 128],
                                         lhsT=xt[:, hh * 128:(hh + 1) * 128],
                                         rhs=ident[:], start=True, stop=True,
                                         is_transpose=True)
                    nc.vector.tensor_copy(
                        R(xT[:, g * 3:(g + 1) * 3, t * 128:(t + 1) * 128]),
                        ps[:, 0:384].rearrange("p (j c) -> p j c", j=3))

            # ---- phase 2: qkT [384, 2048] (chunks {q0q1, k0k1, q2, k2}) ----
            qkA = big.tile([128, T], f32)   # q0 (0:64), q1 (64:128)
            qkB = big.tile([128, T], f32)   # k0, k1
            qkC = big.tile([64, T], f32)    # q2
            qkD = big.tile([64, T], f32)    # k2
            chunks = [(qkA, 0, 128), (qkB, 128, 128), (qkC, 256, 64), (qkD, 320, 64)]
            for mi, (dst, moff, mp) in enumerate(chunks):
                for n in range(4):
                    ns = slice(n * 512, (n + 1) * 512)
                    ps = mmp.tile([128, 512], f32, tag="mm")
                    for k in range(KH):
                        nc.tensor.matmul(ps[0:mp, :], lhsT=R(wqk[:, k, moff:moff + mp]),
                                         rhs=R(xT[:, k, ns]), start=(k == 0), stop=(k == KH - 1))
                    nc.scalar.activation(out=R(dst[:, ns]), in_=ps[0:mp, :], func=Ident,
                                         bias=bqk[0:mp, mi:mi + 1], scale=1.0)

            # ---- phase 2b: v natural [2048, 3*65] with ones column ----
            vsb = big.tile([128, NT, NH_CORE, HD + 1], f32)
            nc.vector.tensor_copy(R(vsb[:, :, :, HD:HD + 1]),
                                  ones128[:, 0:1].to_broadcast((128, NT, NH_CORE, 1)))
            for t in range(NT):
                ps = mmp.tile([128, 256], f32, tag="mm")
                for k in range(KH):
                    nc.tensor.matmul(ps[:], lhsT=R(xT[:, k, t * 128:(t + 1) * 128]),
                                     rhs=R(wv[:, k, :]), start=(k == 0), stop=False)
                nc.tensor.matmul(ps[:], lhsT=R(onesr[:]), rhs=R(bv[:]), start=False, stop=True)
                nc.vector.tensor_copy(R(vsb[:, t, :, 0:HD]),
                                      ps[:, 0:192].rearrange("p (h d) -> p h d", h=NH_CORE))

            # ---- phase 3: attention, query-chunk outer, heads interleaved ----
            # h0 scores use PE rows 0-63, h1 rows 64-127 (adjacent emission ->
            # the PE runs them concurrently); h2 follows on rows 0-63.
            qk_of = [(qkA, 0), (qkA, 64), (qkC, 0)]
            kk_of = [(qkB, 0), (qkB, 64), (qkD, 0)]
            atA = big.tile([128, T], f32)   # A^T for h0 (0:64), h1 (64:128)
            atB = big.tile([64, T], f32)    # h2
            at_of = [(atA, 0), (atA, 64), (atB, 0)]
            qTs = [t_[o:o + HD, :] for (t_, o) in qk_of]
            kTs = [t_[o:o + HD, :] for (t_, o) in kk_of]

            for qc in range(4):
                base = qc * 512
                oaccs = [accp.tile([HD + 1, 512], f32, tag="acc", name=f"oacc{_h}")
                          for _h in range(3)]
                for kk in range(4 * qc + 4):
                    qlo = kk * 128
                    off = max(0, qlo - base)
                    w = 512 - off
                    s2 = mmp.tile([128, 1024], f32, tag="mm")
                    nc.tensor.matmul(s2[:, off:512], lhsT=R(kTs[0][:, qlo:qlo + 128]),
                                     rhs=R(qTs[0][:, base + off:base + 512]),
                                     start=True, stop=True)
                    nc.tensor.matmul(s2[:, 512 + off:1024],
                                     lhsT=R(kTs[1][:, qlo:qlo + 128]),
                                     rhs=R(qTs[1][:, base + off:base + 512]),
                                     start=True, stop=True)
                    s1 = mmp.tile([128, 512], f32, tag="s1", bufs=1)
                    nc.tensor.matmul(s1[:, off:512], lhsT=R(kTs[2][:, qlo:qlo + 128]),
                                     rhs=R(qTs[2][:, base + off:base + 512]),
                                     start=True, stop=True)
                    pt2 = ptp.tile([128, 1024], f32, tag="pt")
                    pt1 = ptp.tile([128, 1024], f32, tag="pt")
                    if off == 0:
                        nc.scalar.activation(out=R(pt2[:]), in_=s2[:], func=Exp,
                                             bias=maskcol[:, kk:kk + 1], scale=1.0)
                    else:
                        nc.scalar.activation(out=R(pt2[:, off:512]), in_=s2[:, off:512],
                                             func=Exp, bias=maskcol[:, kk:kk + 1],
                                             scale=1.0)
                        nc.scalar.activation(out=R(pt2[:, 512 + off:1024]),
                                             in_=s2[:, 512 + off:1024], func=Exp,
                                             bias=maskcol[:, kk:kk + 1], scale=1.0)
                    nc.scalar.activation(out=R(pt1[:, off:512]), in_=s1[:, off:512],
                                         func=Exp, bias=maskcol[:, kk:kk + 1],
                                         scale=1.0)
                    if qlo >= base:
                        # diagonal block: zero keys > query
                        d = off
                        for pt, o2 in ((pt2, 0), (pt2, 512), (pt1, 0)):
                            nc.vector.tensor_tensor(R(pt[:, o2 + d:o2 + d + 128]),
                                                    pt[:, o2 + d:o2 + d + 128],
                                                    tri01[:], mybir.AluOpType.mult)
                    for h, (pt, o2) in enumerate(((pt2, 0), (pt2, 512), (pt1, 0))):
                        nc.tensor.matmul(oaccs[h][0:HD + 1, off:512],
                                         lhsT=R(vsb[:, kk, h, :]),
                                         rhs=R(pt[:, o2 + off:o2 + 512]),
                                         start=(kk == 0), stop=(kk == 4 * qc + 3),
                                         skip_group_check=True)
                # normalize: A^T[:, base:base+512] = out^T[0:64] / s (s = row 64)
                # gpsimd does broadcast + reciprocal-via-divide (SBUF only);
                # DVE does the final fp32r multiply from PSUM.
                for h in range(3):
                    srow = small.tile([1, 512], f32, tag="srow")
                    nc.vector.tensor_copy(srow[:], oaccs[h][HD:HD + 1, :])
                    rrow = small.tile([1, 512], f32, tag="rrow")
                    nc.vector.reciprocal_approx_fast(rrow[:], srow[:])
                    rbs = small.tile([64, 512], f32, tag="rbs")
                    nc.gpsimd.partition_broadcast(rbs[:], rrow[:])
                    at_t, at_o = at_of[h]
                    nc.vector.tensor_tensor(R(at_t[at_o:at_o + HD, base:base + 512]),
                                            oaccs[h][0:HD, :], rbs[:],
                                            mybir.AluOpType.mult)

            # ---- phase 4: projection y = A @ Wp ----
            for t in range(NT):
                ts = slice(t * 128, (t + 1) * 128)
                yt = yout.tile([128, H], f32)
                yps = mmp.tile([128, H], f32, tag="mm")
                for ns in (slice(0, 512), slice(512, 768)):
                    nc.tensor.matmul(yps[:, ns], lhsT=R(atA[:, ts]), rhs=R(wp0[:, ns]),
                                     start=True, stop=False)
                    nc.tensor.matmul(yps[:, ns], lhsT=R(atB[:, ts]), rhs=R(wp1[:, ns]),
                                     start=False, stop=True)
                nc.vector.tensor_copy(yt[:], yps[:])
                nc.sync.dma_start(out=y_d.ap()[ts, :], in_=yt[:])

    nc.compile()
    return nc


def kernel(x, attn_mask, Wqkv, bqkv, Wproj, bproj):
    global last_results
    x = np.asarray(x, dtype=np.float32)
    attn_mask = np.asarray(attn_mask)
    Wqkv = np.asarray(Wqkv, dtype=np.float32)
    bqkv = np.asarray(bqkv, dtype=np.float32)
    Wproj = np.asarray(Wproj, dtype=np.float32)
    bproj = np.asarray(bproj, dtype=np.float32)

    if "nc" not in _cache:
        _cache["nc"] = _build()
    nc = _cache["nc"]

    in_maps = []
    for c in range(8):
        b, g = c // 4, c % 4
        cs = slice(64 * 3 * g, 64 * 3 * g + 192)
        wq = Wqkv[:, 0:768][:, cs] * 0.125
        bq = bqkv[0:768][cs] * 0.125
        wk = Wqkv[:, 768:1536][:, cs]
        bk = bqkv[768:1536][cs]
        wv = Wqkv[:, 1536:2304][:, cs]
        bv = bqkv[1536:2304][cs]
        wqk = np.concatenate([wq[:, 0:128], wk[:, 0:128], wq[:, 128:192], wk[:, 128:192]], axis=1)
        bqk = np.zeros((128, 4), np.float32)
        bqk[:, 0] = bq[0:128]
        bqk[:, 1] = bk[0:128]
        bqk[0:64, 2] = bq[128:192]
        bqk[0:64, 3] = bk[128:192]
        maskcol = np.ascontiguousarray(
            (attn_mask[b].astype(np.float32).reshape(NT, 128).T - 1.0) * 1e9)
        wv_pad = np.zeros((768, 256), np.float32)
        wv_pad[:, 0:192] = wv
        bv_pad = np.zeros((1, 256), np.float32)
        bv_pad[0, 0:192] = bv
        in_maps.append({
            "xb": np.ascontiguousarray(x[b]),
            "wqk": np.ascontiguousarray(wqk),
            "wv": wv_pad,
            "wp": np.ascontiguousarray(Wproj[cs, :]),
            "bqk": bqk,
            "bv": bv_pad,
            "maskcol": maskcol,
        })

    trace = bool(int(os.environ.get("KERNEL_TRACE", "0")))
    res = run_bass_kernel_spmd(nc, in_maps, core_ids=list(range(8)), trace=trace)
    last_results = res

    parts = [res.results[c]["y"] for c in range(8)]
    out = np.stack([
        parts[0] + parts[1] + parts[2] + parts[3],
        parts[4] + parts[5] + parts[6] + parts[7],
    ]).astype(np.float32) + bproj.astype(np.float32)
    return out.astype(np.float32)


# revision 22
# speedup vs baseline: 1.3074x; 1.0632x over previous
"""Causal multi-head self-attention on 8 Trainium2 NeuronCores (Bass/Tile).

Problem (hardcoded shapes): x [2, 2048, 768] f32, 12 heads of dim 64.
    qkv = x @ Wqkv + bqkv ; per-head causal softmax(q k^T / 8) @ v ; out @ Wproj + bproj

Sharding: 8 cores = 2 batches x 4 head-groups (3 heads each). Each core computes
its heads' QKV, attention, and a partial output projection (its rows of Wproj).
Host sums the 4 partial projections per batch and adds bproj.

Per-core device dataflow (all layouts chosen so no on-device transposes of big
activations are ever needed except x -> xT, done as 96 128x128 PE matmuls
against the identity):
  xT[768,2048]   = transpose(x_b)                        (PE + DVE copies)
  qkT[384,2048]  = (Wqk^T x^T), heads on partitions      (PE, K=768 accum)
  v[2048,3*64+1] = x @ Wv, natural layout + ones column  (PE)
  per head, per key-tile kk (S^T layout: keys on partitions, queries free):
    S^T = k_h^T q_h / 8 (scale folded into Wq on host)   (PE, K=64)
    P^T = exp(S^T + causal_tri + key_mask_bias)          (ACT; no max-sub needed,
                                                          |scores| <= O(5) here)
    out_aug^T[65,2048] += v_aug^T P^T                    (PE, K=128; row 64 = colsum s)
  A^T = out^T[0:64] * (1/s broadcast via rank-1 matmul)  (DVE mult)
  y_partial[2048,768] = A @ Wproj_rows                   (PE, K=192)
"""
import os
import numpy as np

import concourse.bass as bass
import concourse.mybir as mybir
import concourse.tile as tile
from concourse import bacc
from concourse.bass_utils import run_bass_kernel_spmd
from concourse.masks import make_identity, make_upper_triangular

f32 = mybir.dt.float32
f32r = mybir.dt.float32r


def R(ap):
    """Bitcast an fp32 AP to float32r for full-rate PE matmuls."""
    return ap.bitcast(f32r)

T = 2048          # sequence length
H = 768           # model dim
NH_CORE = 3       # heads per core
HD = 64           # head dim
NT = T // 128     # 16 seq tiles
KH = H // 128     # 6 contraction chunks for H

_cache = {}
last_results = None


def _build():
    """Build the single-core Tile program (same program on all 8 cores)."""
    nc = bacc.Bacc("TRN2", target_bir_lowering=False, debug=False)

    x_d = nc.dram_tensor("xbT", [H, T], f32, kind="ExternalInput")
    wqk_d = nc.dram_tensor("wqk", [H, 384], f32, kind="ExternalInput")
    wv_d = nc.dram_tensor("wv", [H, 256], f32, kind="ExternalInput")
    wp_d = nc.dram_tensor("wp", [192, H], f32, kind="ExternalInput")
    bqk_d = nc.dram_tensor("bqk", [128, 4], f32, kind="ExternalInput")
    bv_d = nc.dram_tensor("bv", [1, 256], f32, kind="ExternalInput")
    mask_d = nc.dram_tensor("maskcol", [128, NT], f32, kind="ExternalInput")
    y_d = nc.dram_tensor("y", [T, H], f32, kind="ExternalOutput")

    Exp = mybir.ActivationFunctionType.Exp
    Ident = mybir.ActivationFunctionType.Identity

    with tile.TileContext(nc) as tc:
        with (
            tc.tile_pool(name="singles", bufs=1) as singles,
            tc.tile_pool(name="big", bufs=1) as big,
            tc.tile_pool(name="pt", bufs=4) as ptp,
            tc.tile_pool(name="yout", bufs=2) as yout,
            tc.tile_pool(name="small", bufs=2) as small,
            tc.tile_pool(name="wstage", bufs=2) as wstage,
            tc.tile_pool(name="mm", bufs=2, space="PSUM") as mmp,
            tc.tile_pool(name="acc", bufs=3, space="PSUM") as accp,
        ):
            # ---- constants / weights ----
            ident = singles.tile([128, 128], f32)
            make_identity(nc, ident[:])
            tri01 = singles.tile([128, 128], f32)
            make_upper_triangular(nc, tri01[:], val=1.0, diag=True)
            ones = singles.tile([1, 128], f32)
            nc.vector.memset(ones[:], 1.0)
            onesr = singles.tile([1, 128], f32)
            nc.vector.tensor_copy(R(onesr[:]), ones[:])
            ones128 = singles.tile([128, 1], f32)
            nc.vector.memset(ones128[:], 1.0)
            zeros128 = singles.tile([128, 1], f32)
            nc.vector.memset(zeros128[:], 0.0)
            ones_bc = singles.tile([64, 512], f32)
            nc.vector.memset(ones_bc[:], 1.0)

            # DMA-loaded matmul operands must be rounded to fp32r by a
            # compute engine before an fp32r matmul may consume them: stage
            # each load through a scratch tile, round on DVE into the real one.
            def load_rounded(dst, src_ap):
                p = dst.shape[0]
                free = 1
                for d in dst.shape[1:]:
                    free *= d
                st = wstage.tile([128, KH * 384], f32, tag="wstage")
                sv = st[0:p, 0:free]
                if len(dst.shape) == 3:
                    sv = sv.rearrange("p (a m) -> p a m", a=dst.shape[1])
                nc.sync.dma_start(out=sv, in_=src_ap)
                nc.vector.tensor_copy(R(dst[:]), sv)

            wqk = singles.tile([128, KH, 384], f32)
            load_rounded(wqk, wqk_d.ap().rearrange("(a p) m -> p a m", p=128))
            wv = singles.tile([128, KH, 256], f32)
            load_rounded(wv, wv_d.ap().rearrange("(a p) m -> p a m", p=128))
            wp0 = singles.tile([128, H], f32)
            load_rounded(wp0, wp_d.ap()[0:128, :])
            wp1 = singles.tile([64, H], f32)
            load_rounded(wp1, wp_d.ap()[128:192, :])
            bv = singles.tile([1, 256], f32)
            load_rounded(bv, bv_d.ap())
            bqk = singles.tile([128, 4], f32)
            nc.sync.dma_start(out=bqk[:], in_=bqk_d.ap())
            maskcol = singles.tile([128, NT], f32)
            nc.sync.dma_start(out=maskcol[:], in_=mask_d.ap())

            # ---- phase 1: x arrives pre-transposed [768, 2048] from the
            # host shard prep; stage + round to fp32r on DVE ----
            xT = big.tile([128, KH, T], f32)
            for k in range(KH):
                st = wstage.tile([128, KH * 384], f32, tag="wstage")
                nc.sync.dma_start(out=st[:, 0:T],
                                  in_=x_d.ap()[k * 128:(k + 1) * 128, :])
                nc.vector.tensor_copy(R(xT[:, k, :]), st[:, 0:T])

            # ---- phase 2: qkT [384, 2048] (chunks {q0q1, k0k1, q2, k2}) ----
            qkA = big.tile([128, T], f32)   # q0 (0:64), q1 (64:128)
            qkB = big.tile([128, T], f32)   # k0, k1
            qkC = big.tile([64, T], f32)    # q2
            qkD = big.tile([64, T], f32)    # k2
            chunks = [(qkA, 0, 128), (qkB, 128, 128), (qkC, 256, 64), (qkD, 320, 64)]
            for mi, (dst, moff, mp) in enumerate(chunks):
                for n in range(4):
                    ns = slice(n * 512, (n + 1) * 512)
                    ps = mmp.tile([128, 512], f32, tag="mm")
                    for k in range(KH):
                        nc.tensor.matmul(ps[0:mp, :], lhsT=R(wqk[:, k, moff:moff + mp]),
                                         rhs=R(xT[:, k, ns]), start=(k == 0), stop=(k == KH - 1))
                    nc.scalar.activation(out=R(dst[:, ns]), in_=ps[0:mp, :], func=Ident,
                                         bias=bqk[0:mp, mi:mi + 1], scale=1.0)

            # ---- phase 2b: v natural [2048, 3*65] with ones column ----
            vsb = big.tile([128, NT, NH_CORE, HD + 1], f32)
            nc.vector.tensor_copy(R(vsb[:, :, :, HD:HD + 1]),
                                  ones128[:, 0:1].to_broadcast((128, NT, NH_CORE, 1)))
            for t in range(NT):
                ps = mmp.tile([128, 256], f32, tag="mm")
                for k in range(KH):
                    nc.tensor.matmul(ps[:], lhsT=R(xT[:, k, t * 128:(t + 1) * 128]),
                                     rhs=R(wv[:, k, :]), start=(k == 0), stop=False)
                nc.tensor.matmul(ps[:], lhsT=R(onesr[:]), rhs=R(bv[:]), start=False, stop=True)
                nc.vector.tensor_copy(R(vsb[:, t, :, 0:HD]),
                                      ps[:, 0:192].rearrange("p (h d) -> p h d", h=NH_CORE))

            # ---- phase 3: attention, query-chunk outer, heads interleaved ----
            # h0 scores use PE rows 0-63, h1 rows 64-127 (adjacent emission ->
            # the PE runs them concurrently); h2 follows on rows 0-63.
            qk_of = [(qkA, 0), (qkA, 64), (qkC, 0)]
            kk_of = [(qkB, 0), (qkB, 64), (qkD, 0)]
            atA = big.tile([128, T], f32)   # A^T for h0 (0:64), h1 (64:128)
            atB = big.tile([64, T], f32)    # h2
            at_of = [(atA, 0), (atA, 64), (atB, 0)]
            qTs = [t_[o:o + HD, :] for (t_, o) in qk_of]
            kTs = [t_[o:o + HD, :] for (t_, o) in kk_of]

            for qc in range(4):
                base = qc * 512
                oaccs = [accp.tile([HD + 1, 512], f32, tag="acc", name=f"oacc{_h}")
                          for _h in range(3)]
                for kk in range(4 * qc + 4):
                    qlo = kk * 128
                    off = max(0, qlo - base)
                    w = 512 - off
                    s2 = mmp.tile([128, 1024], f32, tag="mm")
                    nc.tensor.matmul(s2[:, off:512], lhsT=R(kTs[0][:, qlo:qlo + 128]),
                                     rhs=R(qTs[0][:, base + off:base + 512]),
                                     start=True, stop=True)
                    nc.tensor.matmul(s2[:, 512 + off:1024],
                                     lhsT=R(kTs[1][:, qlo:qlo + 128]),
                                     rhs=R(qTs[1][:, base + off:base + 512]),
                                     start=True, stop=True)
                    s1 = mmp.tile([128, 512], f32, tag="s1", bufs=1)
                    nc.tensor.matmul(s1[:, off:512], lhsT=R(kTs[2][:, qlo:qlo + 128]),
                                     rhs=R(qTs[2][:, base + off:base + 512]),
                                     start=True, stop=True)
                    pt2 = ptp.tile([128, 1024], f32, tag="pt")
                    pt1 = ptp.tile([128, 1024], f32, tag="pt")
                    if off == 0:
                        nc.scalar.activation(out=R(pt2[:]), in_=s2[:], func=Exp,
                                             bias=maskcol[:, kk:kk + 1], scale=1.0)
                    else:
                        nc.scalar.activation(out=R(pt2[:, off:512]), in_=s2[:, off:512],
                                             func=Exp, bias=maskcol[:, kk:kk + 1],
                                             scale=1.0)
                        nc.scalar.activation(out=R(pt2[:, 512 + off:1024]),
                                             in_=s2[:, 512 + off:1024], func=Exp,
                                             bias=maskcol[:, kk:kk + 1], scale=1.0)
                    nc.scalar.activation(out=R(pt1[:, off:512]), in_=s1[:, off:512],
                                         func=Exp, bias=maskcol[:, kk:kk + 1],
                                         scale=1.0)
                    if qlo >= base:
                        # diagonal block: zero keys > query
                        d = off
                        for pt, o2 in ((pt2, 0), (pt2, 512), (pt1, 0)):
                            nc.vector.tensor_tensor(R(pt[:, o2 + d:o2 + d + 128]),
                                                    pt[:, o2 + d:o2 + d + 128],
                                                    tri01[:], mybir.AluOpType.mult)
                    for h, (pt, o2) in enumerate(((pt2, 0), (pt2, 512), (pt1, 0))):
                        nc.tensor.matmul(oaccs[h][0:HD + 1, off:512],
                                         lhsT=R(vsb[:, kk, h, :]),
                                         rhs=R(pt[:, o2 + off:o2 + 512]),
                                         start=(kk == 0), stop=(kk == 4 * qc + 3),
                                         skip_group_check=True)
                # normalize: A^T[:, base:base+512] = out^T[0:64] / s (s = row 64)
                # gpsimd does broadcast + reciprocal-via-divide (SBUF only);
                # DVE does the final fp32r multiply from PSUM.
                for h in range(3):
                    srow = small.tile([1, 512], f32, tag="srow")
                    nc.vector.tensor_copy(srow[:], oaccs[h][HD:HD + 1, :])
                    rrow = small.tile([1, 512], f32, tag="rrow")
                    nc.vector.reciprocal_approx_fast(rrow[:], srow[:])
                    rbs = small.tile([64, 512], f32, tag="rbs")
                    nc.gpsimd.partition_broadcast(rbs[:], rrow[:])
                    at_t, at_o = at_of[h]
                    nc.vector.tensor_tensor(R(at_t[at_o:at_o + HD, base:base + 512]),
                                            oaccs[h][0:HD, :], rbs[:],
                                            mybir.AluOpType.mult)

            # ---- phase 4: projection y = A @ Wp ----
            for t in range(NT):
                ts = slice(t * 128, (t + 1) * 128)
                yt = yout.tile([128, H], f32)
                yps = mmp.tile([128, H], f32, tag="mm")
                for ns in (slice(0, 512), slice(512, 768)):
                    nc.tensor.matmul(yps[:, ns], lhsT=R(atA[:, ts]), rhs=R(wp0[:, ns]),
                                     start=True, stop=False)
                    nc.tensor.matmul(yps[:, ns], lhsT=R(atB[:, ts]), rhs=R(wp1[:, ns]),
                                     start=False, stop=True)
                nc.vector.tensor_copy(yt[:], yps[:])
                nc.sync.dma_start(out=y_d.ap()[ts, :], in_=yt[:])

    nc.compile()
    return nc


def kernel(x, attn_mask, Wqkv, bqkv, Wproj, bproj):
    global last_results
    x = np.asarray(x, dtype=np.float32)
    attn_mask = np.asarray(attn_mask)
    Wqkv = np.asarray(Wqkv, dtype=np.float32)
    bqkv = np.asarray(bqkv, dtype=np.float32)
    Wproj = np.asarray(Wproj, dtype=np.float32)
    bproj = np.asarray(bproj, dtype=np.float32)

    if "nc" not in _cache:
        _cache["nc"] = _build()
    nc = _cache["nc"]

    in_maps = []
    for c in range(8):
        b, g = c // 4, c % 4
        cs = slice(64 * 3 * g, 64 * 3 * g + 192)
        wq = Wqkv[:, 0:768][:, cs] * 0.125
        bq = bqkv[0:768][cs] * 0.125
        wk = Wqkv[:, 768:1536][:, cs]
        bk = bqkv[768:1536][cs]
        wv = Wqkv[:, 1536:2304][:, cs]
        bv = bqkv[1536:2304][cs]
        wqk = np.concatenate([wq[:, 0:128], wk[:, 0:128], wq[:, 128:192], wk[:, 128:192]], axis=1)
        bqk = np.zeros((128, 4), np.float32)
        bqk[:, 0] = bq[0:128]
        bqk[:, 1] = bk[0:128]
        bqk[0:64, 2] = bq[128:192]
        bqk[0:64, 3] = bk[128:192]
        maskcol = np.ascontiguousarray(
            (attn_mask[b].astype(np.float32).reshape(NT, 128).T - 1.0) * 1e9)
        wv_pad = np.zeros((768, 256), np.float32)
        wv_pad[:, 0:192] = wv
        bv_pad = np.zeros((1, 256), np.float32)
        bv_pad[0, 0:192] = bv
        in_maps.append({
            "xbT": np.ascontiguousarray(x[b].T),
            "wqk": np.ascontiguousarray(wqk),
            "wv": wv_pad,
            "wp": np.ascontiguousarray(Wproj[cs, :]),
            "bqk": bqk,
            "bv": bv_pad,
            "maskcol": maskcol,
        })

    trace = bool(int(os.environ.get("KERNEL_TRACE", "0")))
    res = run_bass_kernel_spmd(nc, in_maps, core_ids=list(range(8)), trace=trace)
    last_results = res

    parts = [res.results[c]["y"] for c in range(8)]
    out = np.stack([
        parts[0] + parts[1] + parts[2] + parts[3],
        parts[4] + parts[5] + parts[6] + parts[7],
    ]).astype(np.float32) + bproj.astype(np.float32)
    return out.astype(np.float32)
